# revision 36
# baseline (speedup 1.0000x reference)
"""BitSelfAttention (relative_key_query position bias) on 8 trn2 cores.

Sharding: core c -> batch b=c//2, head-group g=c%2 (8 heads of 64 dims).
Per core: q/k/v projections for its 512 output dims, then per-head
attention with the Toeplitz relative-position bias realized via a DRAM
round-trip (skewed access patterns) for the Eq/Ek tables.

Layout: scores are computed TRANSPOSED (scoresT[r, l]) so that
 - softmax denominators come free as an extra ones-column in the PV matmul
 - probs never need transposing for PV (expT blocks are the PV stationary)
 - rel_k reads from DRAM are contiguous; rel_q arrives via xbar DMA-transpose
   of a column-reversed Eq table (reversal folded into the host-side deT flip).
"""
import math
from contextlib import ExitStack

import numpy as np

import concourse.bass as bass
import concourse.bacc as bacc
import concourse.tile as tile
from concourse import mybir
from concourse.bass_utils import run_bass_kernel_spmd

B, S, D, H = 4, 1024, 1024, 16
HD = 64
E = 512          # output dims per core (8 heads)
NHC = 8          # heads per core
WD = 2176        # scratch DRAM row width (896 + 1280)
F32 = mybir.dt.float32
F32R = mybir.dt.float32r
BF16 = mybir.dt.bfloat16
FP8 = mybir.dt.float8e4


def build_nc():
    nc = bacc.Bacc()
    hT = nc.declare_dram_parameter("hT", [D, S], BF16, isOutput=False)
    wqT = nc.declare_dram_parameter("wqT", [D, E], BF16, isOutput=False)
    wkT = nc.declare_dram_parameter("wkT", [D, E], BF16, isOutput=False)
    wvT = nc.declare_dram_parameter("wvT", [D, E], BF16, isOutput=False)
    bqr = nc.declare_dram_parameter("bqr", [1, E], BF16, isOutput=False)
    bkr = nc.declare_dram_parameter("bkr", [1, E], BF16, isOutput=False)
    bvr = nc.declare_dram_parameter("bvr", [1, E], BF16, isOutput=False)
    # deT duplicated onto partitions 64..127 so odd heads' K=64 matmuls
    # use lhsT and rhs at the same base partition.
    deTR = nc.declare_dram_parameter("deTR", [128, WD], BF16, isOutput=False)
    deTP = nc.declare_dram_parameter("deTP", [128, WD], BF16, isOutput=False)
    m8t = nc.declare_dram_parameter("m8t", [128, 8], F32, isOutput=False)
    ident = nc.declare_dram_parameter("ident", [128, 128], BF16, isOutput=False)
    out_t = nc.declare_dram_parameter("out", [S, E], F32, isOutput=True)

    # DRAM scratch, fresh per head (no WAR fan-in on reuse)
    eqr = [nc.dram_tensor(f"eqr{i}", [S, WD], BF16) for i in range(8)]
    ekd = [nc.dram_tensor(f"ekd{i}", [S, WD], FP8) for i in range(8)]

    ctx = ExitStack()
    with ctx:
        tc = ctx.enter_context(tile.TileContext(nc))
        consts = ctx.enter_context(tc.tile_pool(name="consts", bufs=1))
        # PSUM budget (8 banks): bandA 2bk x2 + bandB/ctx 1bk x2 + scores 2bk x1
        bandA_ps = ctx.enter_context(tc.tile_pool(name="bandA_ps", bufs=2, space="PSUM"))
        bandB_ps = ctx.enter_context(tc.tile_pool(name="bandB_ps", bufs=1, space="PSUM"))
        score_ps = ctx.enter_context(tc.tile_pool(name="score_ps", bufs=2, space="PSUM"))
        ctx_ps = ctx.enter_context(tc.tile_pool(name="ctx_ps", bufs=1, space="PSUM"))
        band_sb = ctx.enter_context(tc.tile_pool(name="band_sb", bufs=2))
        rel_pool = ctx.enter_context(tc.tile_pool(name="rel_pool", bufs=4))
        expt_pool = ctx.enter_context(tc.tile_pool(name="expt_pool", bufs=12))
        small = ctx.enter_context(tc.tile_pool(name="small", bufs=4))

        # ---- load inputs to SBUF ----
        ht_sb = []
        wq_sb, wk_sb, wv_sb = [], [], []
        for kt in range(8):
            t = consts.tile([128, S], BF16, name=f"ht{kt}")
            nc.sync.dma_start(out=t, in_=hT[kt * 128:(kt + 1) * 128, :])
            ht_sb.append(t)
        for (dst, src, nm) in ((wq_sb, wqT, "wq"), (wk_sb, wkT, "wk"), (wv_sb, wvT, "wv")):
            for kt in range(8):
                t = consts.tile([128, E], BF16, name=f"{nm}{kt}")
                nc.sync.dma_start(out=t, in_=src[kt * 128:(kt + 1) * 128, :])
                dst.append(t)
        detr_sb = consts.tile([128, WD], BF16, name="detr_sb")
        nc.sync.dma_start(out=detr_sb, in_=deTR[:, :])
        detp_sb = consts.tile([128, WD], BF16, name="detp_sb")
        nc.sync.dma_start(out=detp_sb, in_=deTP[:, :])
        m8_sb = consts.tile([128, 8], F32, name="m8_sb")
        nc.sync.dma_start(out=m8_sb, in_=m8t[:, :])
        id_sb = consts.tile([128, 128], BF16, name="id_sb")
        nc.sync.dma_start(out=id_sb, in_=ident[:, :])
        br_sb = {}
        for nm, src in (("bq", bqr), ("bk", bkr), ("bv", bvr)):
            t = consts.tile([1, E], BF16, name=f"{nm}_sb")
            nc.sync.dma_start(out=t, in_=src[:, :])
            br_sb[nm] = t
        ones_sb = consts.tile([1, E], BF16, name="ones_sb")
        nc.vector.memset(ones_sb, 1.0)

        # ---- projections ----
        qT_sb = [consts.tile([128, S], BF16, name=f"qT{et}") for et in range(4)]
        kT_sb = [consts.tile([128, S], BF16, name=f"kT{et}") for et in range(4)]
        for (w_sb, bias, dstl) in ((wq_sb, "bq", qT_sb), (wk_sb, "bk", kT_sb)):
            for et in range(4):
                for ns in range(2):
                    ps = bandA_ps.tile([128, 1024], F32, name="ps_proj", tag="bA")
                    psv = ps[:, 0:512]
                    for kt in range(8):
                        nc.tensor.matmul(
                            psv, w_sb[kt][:, et * 128:(et + 1) * 128],
                            ht_sb[kt][:, ns * 512:(ns + 1) * 512],
                            start=(kt == 0), stop=False)
                    nc.tensor.matmul(
                        psv, br_sb[bias][0:1, et * 128:(et + 1) * 128],
                        ones_sb[0:1, 0:512], start=False, stop=True)
                    if et % 2 == 0:
                        nc.vector.tensor_copy(dstl[et][:, ns * 512:(ns + 1) * 512], psv)
                    else:
                        nc.scalar.copy(dstl[et][:, ns * 512:(ns + 1) * 512], psv)
        # v: natural [S, E] as 8 stile x [128, 8, 65] bf16; col 64 = ones
        v_sb = [consts.tile([128, 8, 65], BF16, name=f"v{st}") for st in range(8)]
        for st in range(8):
            ps = bandA_ps.tile([128, 1024], F32, name="ps_proj", tag="bA")
            psv = ps[:, 0:512]
            for kt in range(8):
                nc.tensor.matmul(
                    psv, ht_sb[kt][:, st * 128:(st + 1) * 128],
                    wv_sb[kt], start=(kt == 0), stop=False)
            nc.tensor.matmul(psv, ones_sb[0:1, 0:128], br_sb["bv"],
                             start=False, stop=True)
            nc.vector.tensor_copy(v_sb[st][:, :, 0:64], psv.rearrange("p (h e) -> p h e", h=8))
            nc.vector.memset(v_sb[st][:, :, 64:65], 1.0)

        out_sb = consts.tile([128, 8, E], F32, name="out_sb")

        # ---- per-head attention ----
        for h in range(NHC):
            et, po = h // 2, 64 * (h % 2)
            EqR, EkD = eqr[h], ekd[h]

            def band(lhs_tile, de_sb, stage, idx, biased):
                base = 896 - 128 * idx
                bA = bandA_ps.tile([128, 1024], F32, name="bA", tag="bA")
                bB = bandB_ps.tile([128, 256], F32, name="bB", tag="bB")
                lhs = lhs_tile
                for wo in (0, 512):
                    nc.tensor.matmul(bA[:, wo:wo + 512],
                                     lhs, de_sb[po:po + 64, base + wo:base + wo + 512],
                                     start=True, stop=True)
                nc.tensor.matmul(bB[:, 0:128], lhs,
                                 de_sb[po:po + 64, base + 1024:base + 1152],
                                 start=True, stop=True)
                if biased:
                    nc.vector.tensor_scalar_add(stage[:, idx, 0:1024], bA,
                                                m8_sb[:, idx:idx + 1])
                    nc.vector.tensor_scalar_add(stage[:, idx, 1024:1152], bB[:, 0:128],
                                                m8_sb[:, idx:idx + 1])
                else:
                    nc.scalar.copy(stage[:, idx, 0:1024], bA)
                    nc.scalar.copy(stage[:, idx, 1024:1152], bB[:, 0:128])

            eq_stage = band_sb.tile([128, 8, 1152], BF16, name="eq_stage", tag="eq_stage")
            ek_stage = band_sb.tile([128, 8, 1152], FP8, name="ek_stage", tag="ek_stage")
            for i in range(8):
                band(qT_sb[et][po:po + 64, i * 128:(i + 1) * 128], detr_sb, eq_stage, i, False)
                band(kT_sb[et][po:po + 64, i * 128:(i + 1) * 128], detp_sb, ek_stage, i, True)
                # per-idx skewed writes: rows l=128*i+p at cols [896-128i,
                # +1152) — each starts as soon as its evacs land, so the
                # table write overlaps the remaining band compute
                for stage, dram in ((eq_stage, EqR), (ek_stage, EkD)):
                    nc.sync.dma_start(
                        out=bass.AP(tensor=dram,
                                    offset=128 * i * WD + 896 - 128 * i,
                                    ap=[[WD, 128], [1, 1152]]),
                        in_=stage[:, i, :])

            expt = []
            for rt in range(8):
                r0 = rt * 128
                rel = rel_pool.tile([128, S], BF16, name="rel", tag="rel")
                nc.sync.dma_start_transpose(
                    out=rel,
                    in_=bass.AP(tensor=EqR, offset=1023 + r0,
                                ap=[[WD - 1, 1024], [1, 128]]))
                nc.gpsimd.dma_start(
                    out=rel,
                    in_=bass.AP(tensor=EkD, offset=(WD - 1) * r0 + 1023,
                                ap=[[WD - 1, 128], [1, 1024]]),
                    accum_op=mybir.AluOpType.add)
                ex = expt_pool.tile([128, S], BF16, name="ex", tag="ex")
                for nh in range(2):
                    sch = score_ps.tile([128, 512], F32, name="sc", tag="sc")
                    nc.tensor.matmul(
                        sch,
                        kT_sb[et][po:po + 64, r0:r0 + 128],
                        qT_sb[et][po:po + 64, nh * 512:(nh + 1) * 512],
                        start=True, stop=False)
                    nc.tensor.matmul(
                        sch, id_sb, rel[:, nh * 512:(nh + 1) * 512],
                        start=False, stop=True)
                    nc.scalar.activation(out=ex[:, nh * 512:(nh + 1) * 512],
                                         in_=sch,
                                         func=mybir.ActivationFunctionType.Exp,
                                         scale=1.0 / math.sqrt(HD))
                expt.append(ex)

            for lt in range(8):
                cx = ctx_ps.tile([128, 65], F32, name="cx", tag="cx")
                for rt in range(8):
                    nc.tensor.matmul(cx, expt[rt][:, lt * 128:(lt + 1) * 128],
                                     v_sb[rt][:, h, :],
                                     start=(rt == 0), stop=(rt == 7))
                rc = small.tile([128, 1], F32, name="rc", tag="rc")
                nc.vector.reciprocal(rc, cx[:, 64:65])
                nc.vector.tensor_scalar_mul(
                    out_sb[:, lt, h * 64:h * 64 + 64], cx[:, 0:64], rc)
            # stream this head's output columns out as soon as PV finishes
            nc.sync.dma_start(
                out=bass.AP(tensor=out_t, offset=h * 64,
                            ap=[[E, 128], [E * 128, 8], [1, 64]]),
                in_=out_sb[:, :, h * 64:h * 64 + 64])
    nc.compile()
    return nc


_NC_CACHE = {}
LAST_RESULT = None
LAST_IN_MAPS = None


def kernel(hidden_states, attention_mask, Wq, bq, Wk, bk, Wv, bv, dist_emb):
    hidden_states = np.asarray(hidden_states, np.float32)
    attention_mask = np.asarray(attention_mask, np.float32)
    Wq, bq = np.asarray(Wq, np.float32), np.asarray(bq, np.float32)
    Wk, bk = np.asarray(Wk, np.float32), np.asarray(bk, np.float32)
    Wv, bv = np.asarray(Wv, np.float32), np.asarray(bv, np.float32)
    dist_emb = np.asarray(dist_emb, np.float32)
    bf = mybir.dt.np(BF16)

    deT = dist_emb.T  # [64, 2047]
    deTP = np.zeros((128, WD), np.float32)
    deTP[0:64, :2047] = deT
    deTP[64:128, :2047] = deT
    deTR = np.zeros((128, WD), np.float32)
    deTR[0:64, :2047] = deT[:, ::-1]
    deTR[64:128, :2047] = deT[:, ::-1]
    ident = np.eye(128).astype(bf)

    if "nc" not in _NC_CACHE:
        _NC_CACHE["nc"] = build_nc()
    nc = _NC_CACHE["nc"]

    in_maps = []
    for c in range(8):
        b, g = c // 2, c % 2
        esl = slice(g * E, (g + 1) * E)
        m8 = (8.0 * attention_mask[b, 0, 0, :]).astype(np.float32)
        in_maps.append({
            "hT": np.ascontiguousarray(hidden_states[b].T).astype(bf),
            "wqT": np.ascontiguousarray(Wq[esl, :].T).astype(bf),
            "wkT": np.ascontiguousarray(Wk[esl, :].T).astype(bf),
            "wvT": np.ascontiguousarray(Wv[esl, :].T).astype(bf),
            "bqr": np.ascontiguousarray(bq[esl][None, :]).astype(bf),
            "bkr": np.ascontiguousarray(bk[esl][None, :]).astype(bf),
            "bvr": np.ascontiguousarray(bv[esl][None, :]).astype(bf),
            "deTR": deTR.astype(bf), "deTP": deTP.astype(bf),
            "m8t": np.ascontiguousarray(m8.reshape(8, 128).T),
            "ident": ident,
        })
    import os as _os
    global LAST_RESULT, LAST_IN_MAPS
    LAST_IN_MAPS = in_maps
    res = run_bass_kernel_spmd(nc, in_maps, core_ids=list(range(8)),
                               trace=bool(_os.environ.get("KTRACE")),
                               tmpdir=_os.environ.get("KTRACE_DIR") or None)
    LAST_RESULT = res
    out = np.empty((B, S, D), np.float32)
    for c in range(8):
        b, g = c // 2, c % 2
        out[b, :, g * E:(g + 1) * E] = res.results[c]["out"]
    return out

